# revision 1
# baseline (speedup 1.0000x reference)
"""Haar DWT (2x2) Trainium2 Bass kernel.

Full input x: (8, 64, 512, 512) fp32. Output: tuple (ll, lh, hl, hh), each
(8, 64, 256, 256) fp32.

Sharding: pure data parallel — core i processes batch element i (64, 512, 512).

Per-core algorithm (per 128-row tile of one 512x512 image):
  - PE matmul with a constant banded stationary matrix V (+-0.5 entries):
    psum = V^T @ X. Rows 0:64 of psum = 0.5*(row-pair sums) P, rows 64:128 =
    0.5*(row-pair diffs) M — the vertical Haar stage including the 0.5 scale.
  - Horizontal stage: ACT copies psum odd columns to SBUF; DVE computes
    tlow = even + odd (rows 0:64 = ll, rows 64:128 = lh) and
    thigh = odd - even (rows 0:64 = hl, rows 64:128 = hh).
  - 4 contiguous DMA stores (64 output rows each).
"""

import sys

if "/opt/trn_rl_repo" not in sys.path:
    sys.path.insert(0, "/opt/trn_rl_repo")

import numpy as np

import concourse.mybir as mybir
from concourse.bacc import Bacc
from concourse.tile import TileContext
from concourse.bass_utils import run_bass_kernel_spmd

N_CORES = 8
C = 64  # images (channels) per core
H = W = 512
OH = OW = 256
F32 = mybir.dt.float32

_cache = {}


def build_nc():
    nc = Bacc("TRN2", target_bir_lowering=False, debug=False, num_devices=N_CORES)
    x = nc.declare_dram_parameter("x", [C, H, W], F32, isOutput=False)
    w = nc.declare_dram_parameter("w", [128, 128], F32, isOutput=False)
    ll = nc.declare_dram_parameter("ll", [C, OH, OW], F32, isOutput=True)
    lh = nc.declare_dram_parameter("lh", [C, OH, OW], F32, isOutput=True)
    hl = nc.declare_dram_parameter("hl", [C, OH, OW], F32, isOutput=True)
    hh = nc.declare_dram_parameter("hh", [C, OH, OW], F32, isOutput=True)

    with TileContext(nc) as tc:
        with (
            tc.tile_pool(name="const", bufs=1) as cpool,
            tc.tile_pool(name="xin", bufs=8) as xpool,
            tc.tile_pool(name="outs", bufs=8) as opool,
            tc.tile_pool(name="psum", bufs=6, space="PSUM") as ppool,
        ):
            vt = cpool.tile([128, 128], F32)
            nc.sync.dma_start(out=vt, in_=w[:, :])
            for c in range(C):
                for rb in range(4):
                    xt = xpool.tile([128, W], F32)
                    nc.sync.dma_start(out=xt, in_=x[c, rb * 128 : (rb + 1) * 128, :])
                    ps = ppool.tile([128, W], F32)
                    nc.tensor.matmul(out=ps, lhsT=vt, rhs=xt, start=True, stop=True)
                    od = opool.tile([128, OW], F32, tag="od")
                    nc.scalar.copy(out=od, in_=ps[:, 1:W:2])
                    tlow = opool.tile([128, OW], F32, tag="tlow")
                    thigh = opool.tile([128, OW], F32, tag="thigh")
                    nc.vector.tensor_add(out=tlow, in0=ps[:, 0:W:2], in1=od)
                    nc.vector.tensor_sub(out=thigh, in0=od, in1=ps[:, 0:W:2])
                    ho = rb * 64
                    nc.sync.dma_start(out=ll[c, ho : ho + 64, :], in_=tlow[0:64, :])
                    nc.sync.dma_start(out=lh[c, ho : ho + 64, :], in_=tlow[64:128, :])
                    nc.sync.dma_start(out=hl[c, ho : ho + 64, :], in_=thigh[0:64, :])
                    nc.sync.dma_start(out=hh[c, ho : ho + 64, :], in_=thigh[64:128, :])
    nc.compile()
    return nc


def make_v():
    v = np.zeros((128, 128), np.float32)
    for m in range(64):
        v[2 * m, m] = 0.5
        v[2 * m + 1, m] = 0.5
        v[2 * m, 64 + m] = -0.5
        v[2 * m + 1, 64 + m] = 0.5
    return v


def get_nc():
    if "nc" not in _cache:
        _cache["nc"] = build_nc()
    return _cache["nc"]


def kernel(x):
    x = np.asarray(x, dtype=np.float32)
    assert x.shape == (N_CORES, C, H, W), x.shape
    nc = get_nc()
    v = make_v()
    in_maps = [{"x": x[i], "w": v} for i in range(N_CORES)]
    res = run_bass_kernel_spmd(nc, in_maps, list(range(N_CORES)))
    outs = []
    for name in ("ll", "lh", "hl", "hh"):
        outs.append(np.stack([res.results[i][name] for i in range(N_CORES)], axis=0))
    return tuple(outs)

